# revision 17
# baseline (speedup 1.0000x reference)
"""Duration-based length regulation (KittenTTS LengthRegulator) on 8 trn2 NeuronCores.

For each batch b (one per core): phoneme t's feature row is repeated
clamp(durations[b,t],1) times along the frame axis; frames are zero-padded to
MAX_LEN = T*15 (outputs arrive pre-zeroed from the runner, so padding rows are
simply left unwritten... except sink rows, sliced off host-side).

Device strategy (per core, batch-parallel across 8 cores):
  Partition p owns phonemes 4p..4p+3 (IPB=4), loaded as one contiguous DMA.
  Cumsum: per-partition DVE scan + strict-lower-triangular ones matmul on PE
  for the cross-partition exclusive prefix (no host-side index math).
  Expand via 15 dma_scatter_add prep/trigger calls (r = 0..14): call r writes
  every phoneme's r-th repeat row (512 one-row tokens, 2KB each) to
  out[exc+r], or to a sink row (MAX_LEN) when r >= dur. prep/trigger defers
  data reads to trigger time so descriptor generation for call k+1 overlaps
  the SDMA drain of call k -- no per-call completion stalls (the v1 kernel's
  bottleneck). Per-token indices avoid the indirect-DMA multi-rank ucode's
  broken source walk.
"""

import sys

import numpy as np

if "/opt/trn_rl_repo" not in sys.path:
    sys.path.insert(0, "/opt/trn_rl_repo")

B, T, D = 8, 512, 512
MAX_DUR = 15
MAX_LEN = T * MAX_DUR  # 7680
P = 128
IPB = T // P  # 4 phonemes per partition
BIG = 1 << 20

_CACHE = {}


def _build_nc():
    from concourse import bass, mybir
    from concourse.bacc import Bacc
    from concourse.tile import TileContext

    f32, i32, i16 = mybir.dt.float32, mybir.dt.int32, mybir.dt.int16
    Alu = mybir.AluOpType

    nc = Bacc()
    feats = nc.declare_dram_parameter("features", [T, D], f32, isOutput=False)
    durs = nc.declare_dram_parameter("durations", [P, IPB], i32, isOutput=False)
    out = nc.declare_dram_parameter("out", [MAX_LEN + 1, D], f32, isOutput=True)
    scratch = nc.dram_tensor("idx_scratch", [16, MAX_DUR * IPB * 8], i32)

    with TileContext(nc) as tc:
        with (
            tc.tile_pool(name="sbuf", bufs=1) as sb,
            tc.tile_pool(name="psum", bufs=1, space="PSUM") as pp,
        ):
            # --- durations [128, 4]: dur[p, i] = durations[4p+i], clamped >= 1
            dur = sb.tile([P, IPB], i32, tag="dur")
            nc.sync.dma_start(out=dur[:], in_=durs[:, :])
            nc.vector.tensor_scalar_max(out=dur[:], in0=dur[:], scalar1=1)

            # --- features: fB[p, i*D:(i+1)*D] = feat[4p+i, :]
            fB = sb.tile([P, IPB * D], f32, tag="fB")
            nc.sync.dma_start(
                out=fB[:],
                in_=feats[:, :].rearrange("(p i) d -> p (i d)", p=P),
            )
            fBv = fB[:].rearrange("p (i d) -> p i d", i=IPB)

            # --- strict-lower-triangular ones [128, 128]: ltri[k, p] = (p > k)
            iota_f = sb.tile([P, P], i32, tag="iota_f")
            nc.gpsimd.iota(out=iota_f[:], pattern=[[1, P]], base=0, channel_multiplier=0)
            iota_p = sb.tile([P, 1], i32, tag="iota_p")
            nc.gpsimd.iota(out=iota_p[:], pattern=[[1, 1]], base=0, channel_multiplier=1)
            ltri = sb.tile([P, P], f32, tag="ltri")
            nc.vector.tensor_tensor(
                out=ltri[:],
                in0=iota_f[:],
                in1=iota_p[:, 0:1].to_broadcast([P, P]),
                op=Alu.is_gt,
            )

            # --- within-partition inclusive scan of the 4 durations
            scan = sb.tile([P, IPB], i32, tag="scan")
            nc.vector.tensor_tensor_scan(
                out=scan[:],
                data0=dur[:],
                data1=dur[:],
                initial=0.0,
                op0=Alu.add,
                op1=Alu.bypass,
            )

            # --- cross-partition exclusive prefix of per-partition totals
            sf = sb.tile([P, 1], f32, tag="sf")
            nc.vector.tensor_copy(out=sf[:], in_=scan[:, IPB - 1 : IPB])
            pre_ps = pp.tile([P, 1], f32, tag="pre_ps")
            nc.tensor.matmul(pre_ps[:], ltri[:], sf[:], start=True, stop=True)
            pre = sb.tile([P, 1], i32, tag="pre")
            nc.vector.tensor_copy(out=pre[:], in_=pre_ps[:])

            # exc[p, i] = global exclusive cumsum at phoneme 4p+i
            exc = sb.tile([P, IPB], i32, tag="exc")
            nc.vector.tensor_tensor(out=exc[:], in0=scan[:], in1=dur[:], op=Alu.subtract)
            nc.vector.tensor_tensor(
                out=exc[:], in0=exc[:], in1=pre[:, 0:1].to_broadcast([P, IPB]), op=Alu.add
            )

            # --- per-call token indices, [128, 60] i32, col r*4+i:
            # idx = exc+r if r < dur else MAX_LEN (sink row)
            idx_all = sb.tile([P, MAX_DUR * IPB], i32, tag="idx_all")
            msk = sb.tile([P, IPB], i32, tag="msk")
            for r in range(MAX_DUR):
                cols = slice(r * IPB, (r + 1) * IPB)
                nc.vector.tensor_scalar(
                    out=msk[:], in0=dur[:], scalar1=r, scalar2=BIG,
                    op0=Alu.is_le, op1=Alu.mult,
                )
                nc.vector.tensor_scalar(
                    out=idx_all[:, cols], in0=msk[:], scalar1=r, scalar2=None, op0=Alu.add
                )
                nc.vector.tensor_tensor(
                    out=idx_all[:, cols], in0=idx_all[:, cols], in1=exc[:], op=Alu.add
                )
                nc.vector.tensor_scalar_min(
                    out=idx_all[:, cols], in0=idx_all[:, cols], scalar1=MAX_LEN
                )

            # --- DRAM round-trip to the scatter_add idx layout: token j of call
            # r is (p = j%128, i = j//128) with its int16 idx at
            # [channel j%16, col j//16] -> col (i*8 + p//16), replicated to all
            # eight 16-partition groups (each Q7 core reads its own group).
            nc.sync.dma_start(
                out=scratch[:, :].rearrange("q (ri w) -> w q ri", w=8),
                in_=idx_all[:],
            )
            idx32 = sb.tile([P, MAX_DUR * IPB * 8], i32, tag="idx32")
            for g in range(8):
                nc.sync.dma_start(
                    out=idx32[16 * g : 16 * (g + 1), :], in_=scratch[:, :]
                )
            idx16 = sb.tile([P, MAX_DUR * IPB * 8], i16, tag="idx16")
            nc.vector.tensor_copy(out=idx16[:], in_=idx32[:])

            # --- 15 scatter preps + triggers; transfers overlap later desc-gen
            dma_sem = nc.alloc_semaphore("swdge_dma")
            for r in range(MAX_DUR):
                nc.gpsimd.dma_scatter_add(
                    out[:, :],
                    fBv,
                    idx16[:, r * 32 : (r + 1) * 32],
                    T,
                    T,
                    D,
                    prepare_only=True,
                    sem=dma_sem,
                )
                nc.gpsimd.trigger_dma(count=None)
            nc.gpsimd.wait_ge(dma_sem, MAX_DUR * 16)

    nc.compile()
    return nc


def _get_nc():
    if "nc" not in _CACHE:
        _CACHE["nc"] = _build_nc()
    return _CACHE["nc"]


def _run(features, durations, trace=False):
    """features (B,T,D) f32, durations (B,T) i32 -> (out (B,MAX_LEN,D) f32, BassKernelResults)."""
    from concourse.bass_utils import run_bass_kernel_spmd

    nc = _get_nc()
    in_maps = []
    for b in range(B):
        in_maps.append(
            {
                "features": np.ascontiguousarray(features[b]),
                "durations": np.ascontiguousarray(durations[b].reshape(P, IPB)),
            }
        )
    kwargs = {}
    if trace:
        kwargs = dict(trace=True, trace_cores=list(range(B)), stitch_traces=False)
    res = run_bass_kernel_spmd(nc, in_maps, core_ids=list(range(B)), **kwargs)
    outs = np.stack([res.results[b]["out"][:MAX_LEN] for b in range(B)])
    return outs.astype(np.float32, copy=False), res


def kernel(features, durations):
    features = np.asarray(features, dtype=np.float32)
    durations = np.asarray(durations, dtype=np.int32)
    outs, _ = _run(features, durations, trace=False)
    return outs


if __name__ == "__main__":
    feats = np.random.randn(B, T, D).astype(np.float32)
    durs = np.random.randint(0, 16, size=(B, T)).astype(np.int32)
    out = kernel(feats, durs)
    print("out", out.shape, out.dtype)


# revision 19
# speedup vs baseline: 5.8712x; 5.8712x over previous
"""Duration-based length regulation (KittenTTS LengthRegulator) on 8 trn2 NeuronCores.

For each batch b (one per core): phoneme t's feature row is repeated
clamp(durations[b,t],1) times along the frame axis; frames are zero-padded to
MAX_LEN = T*15 (outputs arrive pre-zeroed from the runner, so padding rows are
simply left unwritten... except sink rows, sliced off host-side).

Device strategy (per core, batch-parallel across 8 cores):
  Partition p owns phonemes 4p..4p+3 (IPB=4), loaded as one contiguous DMA.
  Cumsum: per-partition DVE scan + strict-lower-triangular ones matmul on PE
  for the cross-partition exclusive prefix (no host-side index math).
  Expand via 16 single-rank indirect scatters ({8,4,2,1}-row blocks x 4
  phoneme slots, binary decomposition of dur; masked slots pushed past
  bounds_check and skipped), issued in a RAW bass region after TileContext:
  without Tile's conservative WAW waits between them, Q7 streams descriptor
  generation back-to-back and the SDMA drains overlap -- the v1 kernel lost
  ~50us to per-call completion stalls here. Write traffic ~= live bytes.
"""

import sys

import numpy as np

if "/opt/trn_rl_repo" not in sys.path:
    sys.path.insert(0, "/opt/trn_rl_repo")

B, T, D = 8, 512, 512
MAX_DUR = 15
MAX_LEN = T * MAX_DUR  # 7680
P = 128
IPB = T // P  # 4 phonemes per partition
SBLK = [8, 4, 2, 1]  # binary block sizes
OOB = 1 << 20

_CACHE = {}


def _build_nc():
    from concourse import bass, mybir
    from concourse.bacc import Bacc
    from concourse.tile import TileContext

    f32, i32, i16 = mybir.dt.float32, mybir.dt.int32, mybir.dt.int16
    Alu = mybir.AluOpType

    nc = Bacc()
    feats = nc.declare_dram_parameter("features", [T, D], f32, isOutput=False)
    durs = nc.declare_dram_parameter("durations", [P, IPB], i32, isOutput=False)
    out = nc.declare_dram_parameter("out", [MAX_LEN, D], f32, isOutput=True)
    
    with TileContext(nc) as tc:
        with (
            tc.tile_pool(name="sbuf", bufs=1) as sb,
            tc.tile_pool(name="psum", bufs=1, space="PSUM") as pp,
        ):
            # --- durations [128, 4]: dur[p, i] = durations[4p+i], clamped >= 1
            dur = sb.tile([P, IPB], i32, tag="dur")
            nc.sync.dma_start(out=dur[:], in_=durs[:, :])
            nc.vector.tensor_scalar_max(out=dur[:], in0=dur[:], scalar1=1)

            # --- features: fB[p, i*D:(i+1)*D] = feat[4p+i, :]
            fB = sb.tile([P, IPB * D], f32, tag="fB")
            nc.sync.dma_start(
                out=fB[:],
                in_=feats[:, :].rearrange("(p i) d -> p (i d)", p=P),
            )
            fBv = fB[:].rearrange("p (i d) -> p i d", i=IPB)

            # --- strict-lower-triangular ones [128, 128]: ltri[k, p] = (p > k)
            iota_f = sb.tile([P, P], i32, tag="iota_f")
            nc.gpsimd.iota(out=iota_f[:], pattern=[[1, P]], base=0, channel_multiplier=0)
            iota_p = sb.tile([P, 1], i32, tag="iota_p")
            nc.gpsimd.iota(out=iota_p[:], pattern=[[1, 1]], base=0, channel_multiplier=1)
            ltri = sb.tile([P, P], f32, tag="ltri")
            nc.vector.tensor_tensor(
                out=ltri[:],
                in0=iota_f[:],
                in1=iota_p[:, 0:1].to_broadcast([P, P]),
                op=Alu.is_gt,
            )

            # --- within-partition inclusive scan of the 4 durations
            scan = sb.tile([P, IPB], i32, tag="scan")
            nc.vector.tensor_tensor_scan(
                out=scan[:],
                data0=dur[:],
                data1=dur[:],
                initial=0.0,
                op0=Alu.add,
                op1=Alu.bypass,
            )

            # --- cross-partition exclusive prefix of per-partition totals
            sf = sb.tile([P, 1], f32, tag="sf")
            nc.vector.tensor_copy(out=sf[:], in_=scan[:, IPB - 1 : IPB])
            pre_ps = pp.tile([P, 1], f32, tag="pre_ps")
            nc.tensor.matmul(pre_ps[:], ltri[:], sf[:], start=True, stop=True)
            pre = sb.tile([P, 1], i32, tag="pre")
            nc.vector.tensor_copy(out=pre[:], in_=pre_ps[:])

            # exc[p, i] = global exclusive cumsum at phoneme 4p+i
            exc = sb.tile([P, IPB], i32, tag="exc")
            nc.vector.tensor_tensor(out=exc[:], in0=scan[:], in1=dur[:], op=Alu.subtract)
            nc.vector.tensor_tensor(
                out=exc[:], in0=exc[:], in1=pre[:, 0:1].to_broadcast([P, IPB]), op=Alu.add
            )

            # --- scatter offsets [128, 16], col si*4+i: binary block of size
            # s for phoneme 4p+i at row exc + (dur & ~(2s-1)); masked to OOB
            # unless (dur & s)
            offs_h = nc.alloc_sbuf_tensor("offs_raw", [P, 16], i32)
            offs = offs_h[:]
            hi = sb.tile([P, IPB], i32, tag="hi")
            msk = sb.tile([P, IPB], i32, tag="msk")
            for si, s_ in enumerate(SBLK):
                cols = slice(si * IPB, (si + 1) * IPB)
                nc.vector.tensor_scalar(
                    out=hi[:], in0=dur[:], scalar1=-(2 * s_), scalar2=None,
                    op0=Alu.bitwise_and,
                )
                nc.vector.tensor_tensor(out=offs[:, cols], in0=hi[:], in1=exc[:], op=Alu.add)
                nc.vector.tensor_scalar(
                    out=msk[:], in0=dur[:], scalar1=s_, scalar2=None, op0=Alu.bitwise_and
                )
                nc.vector.tensor_scalar(
                    out=msk[:], in0=msk[:], scalar1=0, scalar2=OOB, op0=Alu.is_equal, op1=Alu.mult
                )
                nc.vector.tensor_tensor(
                    out=offs[:, cols], in0=offs[:, cols], in1=msk[:], op=Alu.add
                )

            # --- rep[p, i, c, :] = feat[4p+i, :] for c = 0..7 (8 copies; the
            # s-row scatter reads the first s). Doubling builds, split across
            # DVE / ACT / Pool by phoneme slot.
            rep_h = nc.alloc_sbuf_tensor("rep_raw", [P, IPB, 8, D], f32)
            rep = rep_h[:]
            for eng, lo, hi_ in ((nc.vector, 0, 2), (nc.scalar, 2, 3), (nc.gpsimd, 3, 4)):
                cp = eng.copy if eng is nc.scalar else eng.tensor_copy
                cp(out=rep[:, lo:hi_, 0, :], in_=fBv[:, lo:hi_, :])
                cp(out=rep[:, lo:hi_, 1, :], in_=rep[:, lo:hi_, 0, :])
                cp(out=rep[:, lo:hi_, 2:4, :], in_=rep[:, lo:hi_, 0:2, :])
                cp(out=rep[:, lo:hi_, 4:8, :], in_=rep[:, lo:hi_, 0:4, :])

    # --- RAW region: 16 scatters with no inter-call waits. The TileContext
    # epilogue barrier guarantees offsets and rep are ready.
    scat_sem = nc.alloc_semaphore("scat")
    bregs = {s_: nc.gpsimd.to_reg(MAX_LEN - s_) for s_ in SBLK}
    for si, s_ in enumerate(SBLK):
        for i in range(IPB):
            c = si * IPB + i
            nc.gpsimd.indirect_dma_start(
                out=out[:, :],
                out_offset=bass.IndirectOffsetOnAxis(ap=offs[:, c : c + 1], axis=0),
                in_=rep[:, i, 0:s_, :].rearrange("p c d -> p (c d)"),
                in_offset=None,
                bounds_check=bregs[s_],
                oob_is_err=False,
            ).then_inc(scat_sem, 16)
    nc.gpsimd.wait_ge(scat_sem, 16 * 16)

    nc.compile()
    return nc


def _get_nc():
    if "nc" not in _CACHE:
        _CACHE["nc"] = _build_nc()
    return _CACHE["nc"]


def _run(features, durations, trace=False):
    """features (B,T,D) f32, durations (B,T) i32 -> (out (B,MAX_LEN,D) f32, BassKernelResults)."""
    from concourse.bass_utils import run_bass_kernel_spmd

    nc = _get_nc()
    in_maps = []
    for b in range(B):
        in_maps.append(
            {
                "features": np.ascontiguousarray(features[b]),
                "durations": np.ascontiguousarray(durations[b].reshape(P, IPB)),
            }
        )
    kwargs = {}
    if trace:
        kwargs = dict(trace=True, trace_cores=list(range(B)), stitch_traces=False)
    res = run_bass_kernel_spmd(nc, in_maps, core_ids=list(range(B)), **kwargs)
    outs = np.stack([res.results[b]["out"] for b in range(B)])
    return outs.astype(np.float32, copy=False), res


def kernel(features, durations):
    features = np.asarray(features, dtype=np.float32)
    durations = np.asarray(durations, dtype=np.int32)
    outs, _ = _run(features, durations, trace=False)
    return outs


if __name__ == "__main__":
    feats = np.random.randn(B, T, D).astype(np.float32)
    durs = np.random.randint(0, 16, size=(B, T)).astype(np.int32)
    out = kernel(feats, durs)
    print("out", out.shape, out.dtype)


# revision 21
# speedup vs baseline: 6.7906x; 1.1566x over previous
"""Duration-based length regulation (KittenTTS LengthRegulator) on 8 trn2 NeuronCores.

For each batch b (one per core): phoneme t's feature row is repeated
clamp(durations[b,t],1) times along the frame axis; frames are zero-padded to
MAX_LEN = T*15 (outputs arrive pre-zeroed from the runner, so padding rows are
simply left unwritten... except sink rows, sliced off host-side).

Device strategy (per core, batch-parallel across 8 cores):
  Partition p owns phonemes 4p..4p+3 (IPB=4), loaded as one contiguous DMA.
  Cumsum: per-partition DVE scan + strict-lower-triangular ones matmul on PE
  for the cross-partition exclusive prefix (no host-side index math).
  Expand via 20 single-rank indirect scatters ({4,4,4,2,1}-row blocks x 4
  phoneme slots, dur = 4*q + binary tail; masked slots pushed past
  bounds_check and skipped), issued in a RAW bass region after TileContext:
  without Tile's conservative WAW waits between them, Q7 streams descriptor
  generation back-to-back and the SDMA drains overlap -- the v1 kernel lost
  ~50us to per-call completion stalls here. Write traffic ~= live bytes.
"""

import sys

import numpy as np

if "/opt/trn_rl_repo" not in sys.path:
    sys.path.insert(0, "/opt/trn_rl_repo")

B, T, D = 8, 512, 512
MAX_DUR = 15
MAX_LEN = T * MAX_DUR  # 7680
P = 128
IPB = T // P  # 4 phonemes per partition
BLKS = [(4, 0), (4, 4), (4, 8), (2, None), (1, None)]  # (rows, m-offset)
OOB = 1 << 20

_CACHE = {}


def _build_nc():
    from concourse import bass, mybir
    from concourse.bacc import Bacc
    from concourse.tile import TileContext

    f32, i32, i16 = mybir.dt.float32, mybir.dt.int32, mybir.dt.int16
    Alu = mybir.AluOpType

    nc = Bacc()
    feats = nc.declare_dram_parameter("features", [T, D], f32, isOutput=False)
    durs = nc.declare_dram_parameter("durations", [P, IPB], i32, isOutput=False)
    out = nc.declare_dram_parameter("out", [MAX_LEN, D], f32, isOutput=True)
    
    with TileContext(nc) as tc:
        with (
            tc.tile_pool(name="sbuf", bufs=1) as sb,
            tc.tile_pool(name="psum", bufs=1, space="PSUM") as pp,
        ):
            # --- durations [128, 4]: dur[p, i] = durations[4p+i], clamped >= 1
            dur = sb.tile([P, IPB], i32, tag="dur")
            nc.sync.dma_start(out=dur[:], in_=durs[:, :])
            nc.vector.tensor_scalar_max(out=dur[:], in0=dur[:], scalar1=1)

            # --- features: fB[p, i*D:(i+1)*D] = feat[4p+i, :]
            fB_h = nc.alloc_sbuf_tensor("fB_raw", [P, IPB * D], f32)
            fB = fB_h
            nc.sync.dma_start(
                out=fB[:],
                in_=feats[:, :].rearrange("(p i) d -> p (i d)", p=P),
            )
            fBv = fB[:].rearrange("p (i d) -> p i d", i=IPB)

            # --- strict-lower-triangular ones [128, 128]: ltri[k, p] = (p > k)
            iota_f = sb.tile([P, P], i32, tag="iota_f")
            nc.gpsimd.iota(out=iota_f[:], pattern=[[1, P]], base=0, channel_multiplier=0)
            iota_p = sb.tile([P, 1], i32, tag="iota_p")
            nc.gpsimd.iota(out=iota_p[:], pattern=[[1, 1]], base=0, channel_multiplier=1)
            ltri = sb.tile([P, P], f32, tag="ltri")
            nc.vector.tensor_tensor(
                out=ltri[:],
                in0=iota_f[:],
                in1=iota_p[:, 0:1].to_broadcast([P, P]),
                op=Alu.is_gt,
            )

            # --- within-partition inclusive scan of the 4 durations
            scan = sb.tile([P, IPB], i32, tag="scan")
            nc.vector.tensor_tensor_scan(
                out=scan[:],
                data0=dur[:],
                data1=dur[:],
                initial=0.0,
                op0=Alu.add,
                op1=Alu.bypass,
            )

            # --- cross-partition exclusive prefix of per-partition totals
            sf = sb.tile([P, 1], f32, tag="sf")
            nc.vector.tensor_copy(out=sf[:], in_=scan[:, IPB - 1 : IPB])
            pre_ps = pp.tile([P, 1], f32, tag="pre_ps")
            nc.tensor.matmul(pre_ps[:], ltri[:], sf[:], start=True, stop=True)
            pre = sb.tile([P, 1], i32, tag="pre")
            nc.vector.tensor_copy(out=pre[:], in_=pre_ps[:])

            # exc[p, i] = global exclusive cumsum at phoneme 4p+i
            exc = sb.tile([P, IPB], i32, tag="exc")
            nc.vector.tensor_tensor(out=exc[:], in0=scan[:], in1=dur[:], op=Alu.subtract)
            nc.vector.tensor_tensor(
                out=exc[:], in0=exc[:], in1=pre[:, 0:1].to_broadcast([P, IPB]), op=Alu.add
            )

            # --- scatter offsets [128, 20], col ci*4+i. Classes:
            # m-blocks (4 rows at exc+4m, live iff dur >= 4(m+1)) then the
            # binary tail: 2-row at exc+(dur&~3) iff dur&2, 1-row at
            # exc+(dur&~1) iff dur&1. Masked cols pushed to OOB.
            offs_h = nc.alloc_sbuf_tensor("offs_raw", [P, 20], i32)
            offs = offs_h[:]
            hi = sb.tile([P, IPB], i32, tag="hi")
            msk = sb.tile([P, IPB], i32, tag="msk")
            for ci, (s_, moff) in enumerate(BLKS):
                cols = slice(ci * IPB, (ci + 1) * IPB)
                if moff is not None:  # 4-row block m = moff//4
                    nc.vector.tensor_scalar(
                        out=msk[:], in0=dur[:], scalar1=4 + moff, scalar2=OOB - moff,
                        op0=Alu.is_lt, op1=Alu.mult,
                    )
                    nc.vector.tensor_scalar(
                        out=msk[:], in0=msk[:], scalar1=moff, scalar2=None, op0=Alu.add
                    )
                    nc.vector.tensor_tensor(
                        out=offs[:, cols], in0=msk[:], in1=exc[:], op=Alu.add
                    )
                else:  # binary tail block of s_ rows
                    nc.vector.tensor_scalar(
                        out=hi[:], in0=dur[:], scalar1=-(2 * s_), scalar2=None,
                        op0=Alu.bitwise_and,
                    )
                    nc.vector.tensor_tensor(
                        out=offs[:, cols], in0=hi[:], in1=exc[:], op=Alu.add
                    )
                    nc.vector.tensor_scalar(
                        out=msk[:], in0=dur[:], scalar1=s_, scalar2=None,
                        op0=Alu.bitwise_and,
                    )
                    nc.vector.tensor_scalar(
                        out=msk[:], in0=msk[:], scalar1=0, scalar2=OOB,
                        op0=Alu.is_equal, op1=Alu.mult,
                    )
                    nc.vector.tensor_tensor(
                        out=offs[:, cols], in0=offs[:, cols], in1=msk[:], op=Alu.add
                    )

            # --- rep[p, i, c, :] = feat[4p+i, :] for c = 0..3 (4 copies; the
            # 4/2-row scatters read the first 4/2; 1-row reads fB directly).
            # Doubling builds, split across DVE / ACT / Pool by phoneme slot.
            rep_h = nc.alloc_sbuf_tensor("rep_raw", [P, IPB, 4, D], f32)
            rep = rep_h[:]
            for eng, lo, hh in ((nc.vector, 0, 2), (nc.scalar, 2, 3), (nc.gpsimd, 3, 4)):
                cp = eng.copy if eng is nc.scalar else eng.tensor_copy
                cp(out=rep[:, lo:hh, 0, :], in_=fBv[:, lo:hh, :])
                cp(out=rep[:, lo:hh, 1, :], in_=rep[:, lo:hh, 0, :])
                cp(out=rep[:, lo:hh, 2:4, :], in_=rep[:, lo:hh, 0:2, :])

    # --- RAW region: 20 scatters with no inter-call waits. The TileContext
    # epilogue barrier guarantees offsets, fB and rep are ready.
    scat_sem = nc.alloc_semaphore("scat")
    bregs = {s_: nc.gpsimd.to_reg(MAX_LEN - s_) for s_ in (4, 2, 1)}
    fBr = fB[:].rearrange("p (i d) -> p i d", i=IPB)
    for ci, (s_, moff) in enumerate(BLKS):
        for i in range(IPB):
            c = ci * IPB + i
            if s_ == 1:
                src_ap = fBr[:, i, :]
            else:
                src_ap = rep[:, i, 0:s_, :].rearrange("p c d -> p (c d)")
            nc.gpsimd.indirect_dma_start(
                out=out[:, :],
                out_offset=bass.IndirectOffsetOnAxis(ap=offs[:, c : c + 1], axis=0),
                in_=src_ap,
                in_offset=None,
                bounds_check=bregs[s_],
                oob_is_err=False,
            ).then_inc(scat_sem, 16)
    nc.gpsimd.wait_ge(scat_sem, 20 * 16)

    nc.compile()
    return nc


def _get_nc():
    if "nc" not in _CACHE:
        _CACHE["nc"] = _build_nc()
    return _CACHE["nc"]


def _run(features, durations, trace=False):
    """features (B,T,D) f32, durations (B,T) i32 -> (out (B,MAX_LEN,D) f32, BassKernelResults)."""
    from concourse.bass_utils import run_bass_kernel_spmd

    nc = _get_nc()
    in_maps = []
    for b in range(B):
        in_maps.append(
            {
                "features": np.ascontiguousarray(features[b]),
                "durations": np.ascontiguousarray(durations[b].reshape(P, IPB)),
            }
        )
    kwargs = {}
    if trace:
        kwargs = dict(trace=True, trace_cores=list(range(B)), stitch_traces=False)
    res = run_bass_kernel_spmd(nc, in_maps, core_ids=list(range(B)), **kwargs)
    outs = np.stack([res.results[b]["out"] for b in range(B)])
    return outs.astype(np.float32, copy=False), res


def kernel(features, durations):
    features = np.asarray(features, dtype=np.float32)
    durations = np.asarray(durations, dtype=np.int32)
    outs, _ = _run(features, durations, trace=False)
    return outs


if __name__ == "__main__":
    feats = np.random.randn(B, T, D).astype(np.float32)
    durs = np.random.randint(0, 16, size=(B, T)).astype(np.int32)
    out = kernel(feats, durs)
    print("out", out.shape, out.dtype)


# revision 22
# speedup vs baseline: 7.0915x; 1.0443x over previous
"""Duration-based length regulation (KittenTTS LengthRegulator) on 8 trn2 NeuronCores.

For each batch b (one per core): phoneme t's feature row is repeated
clamp(durations[b,t],1) times along the frame axis; frames are zero-padded to
MAX_LEN = T*15 (outputs arrive pre-zeroed from the runner, so padding rows are
simply left unwritten... except sink rows, sliced off host-side).

Device strategy (per core, batch-parallel across 8 cores):
  Partition p owns phonemes 4p..4p+3 (IPB=4), loaded as one contiguous DMA.
  Cumsum: per-partition DVE scan + strict-lower-triangular ones matmul on PE
  for the cross-partition exclusive prefix (no host-side index math).
  Expand via 20 single-rank indirect scatters ({4,4,4,2,1}-row blocks x 4
  phoneme slots, dur = 4*q + binary tail; masked slots pushed past
  bounds_check and skipped), issued in a RAW bass region after TileContext:
  without Tile's conservative WAW waits between them, Q7 streams descriptor
  generation back-to-back and the SDMA drains overlap -- the v1 kernel lost
  ~50us to per-call completion stalls here. Write traffic ~= live bytes.
"""

import sys

import numpy as np

if "/opt/trn_rl_repo" not in sys.path:
    sys.path.insert(0, "/opt/trn_rl_repo")

B, T, D = 8, 512, 512
MAX_DUR = 15
MAX_LEN = T * MAX_DUR  # 7680
P = 128
IPB = T // P  # 4 phonemes per partition
BLKS = [(4, 0), (4, 4), (4, 8), (2, None), (1, None)]  # (rows, m-offset)
OOB = 1 << 20

_CACHE = {}


def _build_nc():
    from concourse import bass, mybir
    from concourse.bacc import Bacc
    from concourse.tile import TileContext

    f32, i32, i16 = mybir.dt.float32, mybir.dt.int32, mybir.dt.int16
    Alu = mybir.AluOpType

    nc = Bacc()
    feats = nc.declare_dram_parameter("features", [T, D], f32, isOutput=False)
    durs = nc.declare_dram_parameter("durations", [P, IPB], i32, isOutput=False)
    out = nc.declare_dram_parameter("out", [MAX_LEN, D], f32, isOutput=True)
    
    with TileContext(nc) as tc:
        with (
            tc.tile_pool(name="sbuf", bufs=1) as sb,
            tc.tile_pool(name="psum", bufs=1, space="PSUM") as pp,
        ):
            # --- features: fB[p, i*D:(i+1)*D] = feat[4p+i, :]
            fB_h = nc.alloc_sbuf_tensor("fB_raw", [P, IPB * D], f32)
            fB = fB_h
            nc.sync.dma_start(
                out=fB[:],
                in_=feats[:, :].rearrange("(p i) d -> p (i d)", p=P),
            )
            fBv = fB[:].rearrange("p (i d) -> p i d", i=IPB)

            # --- durations [128, 4]: dur[p, i] = durations[4p+i], clamped >= 1
            dur = sb.tile([P, IPB], i32, tag="dur")
            nc.sync.dma_start(out=dur[:], in_=durs[:, :])
            nc.vector.tensor_scalar_max(out=dur[:], in0=dur[:], scalar1=1)

            # --- strict-lower-triangular ones [128, 128]: ltri[k, p] = (p > k)
            iota_f = sb.tile([P, P], i32, tag="iota_f")
            nc.gpsimd.iota(out=iota_f[:], pattern=[[1, P]], base=0, channel_multiplier=0)
            iota_p = sb.tile([P, 1], i32, tag="iota_p")
            nc.gpsimd.iota(out=iota_p[:], pattern=[[1, 1]], base=0, channel_multiplier=1)
            ltri = sb.tile([P, P], f32, tag="ltri")
            nc.vector.tensor_tensor(
                out=ltri[:],
                in0=iota_f[:],
                in1=iota_p[:, 0:1].to_broadcast([P, P]),
                op=Alu.is_gt,
            )

            # --- within-partition inclusive scan of the 4 durations
            scan = sb.tile([P, IPB], i32, tag="scan")
            nc.vector.tensor_tensor_scan(
                out=scan[:],
                data0=dur[:],
                data1=dur[:],
                initial=0.0,
                op0=Alu.add,
                op1=Alu.bypass,
            )

            # --- cross-partition exclusive prefix of per-partition totals
            sf = sb.tile([P, 1], f32, tag="sf")
            nc.vector.tensor_copy(out=sf[:], in_=scan[:, IPB - 1 : IPB])
            pre_ps = pp.tile([P, 1], f32, tag="pre_ps")
            nc.tensor.matmul(pre_ps[:], ltri[:], sf[:], start=True, stop=True)
            pre = sb.tile([P, 1], i32, tag="pre")
            nc.vector.tensor_copy(out=pre[:], in_=pre_ps[:])

            # exc[p, i] = global exclusive cumsum at phoneme 4p+i
            exc = sb.tile([P, IPB], i32, tag="exc")
            nc.vector.tensor_tensor(out=exc[:], in0=scan[:], in1=dur[:], op=Alu.subtract)
            nc.vector.tensor_tensor(
                out=exc[:], in0=exc[:], in1=pre[:, 0:1].to_broadcast([P, IPB]), op=Alu.add
            )

            # --- scatter offsets [128, 20], col ci*4+i. Classes:
            # m-blocks (4 rows at exc+4m, live iff dur >= 4(m+1)) then the
            # binary tail: 2-row at exc+(dur&~3) iff dur&2, 1-row at
            # exc+(dur&~1) iff dur&1. Masked cols pushed to OOB.
            offs_h = nc.alloc_sbuf_tensor("offs_raw", [P, 20], i32)
            offs = offs_h[:]
            hi = sb.tile([P, IPB], i32, tag="hi")
            msk = sb.tile([P, IPB], i32, tag="msk")
            for ci, (s_, moff) in enumerate(BLKS):
                cols = slice(ci * IPB, (ci + 1) * IPB)
                if moff is not None:  # 4-row block m = moff//4
                    nc.vector.tensor_scalar(
                        out=msk[:], in0=dur[:], scalar1=4 + moff, scalar2=OOB - moff,
                        op0=Alu.is_lt, op1=Alu.mult,
                    )
                    nc.vector.tensor_scalar(
                        out=msk[:], in0=msk[:], scalar1=moff, scalar2=None, op0=Alu.add
                    )
                    nc.vector.tensor_tensor(
                        out=offs[:, cols], in0=msk[:], in1=exc[:], op=Alu.add
                    )
                else:  # binary tail block of s_ rows
                    nc.vector.tensor_scalar(
                        out=hi[:], in0=dur[:], scalar1=-(2 * s_), scalar2=None,
                        op0=Alu.bitwise_and,
                    )
                    nc.vector.tensor_tensor(
                        out=offs[:, cols], in0=hi[:], in1=exc[:], op=Alu.add
                    )
                    nc.vector.tensor_scalar(
                        out=msk[:], in0=dur[:], scalar1=s_, scalar2=None,
                        op0=Alu.bitwise_and,
                    )
                    nc.vector.tensor_scalar(
                        out=msk[:], in0=msk[:], scalar1=0, scalar2=OOB,
                        op0=Alu.is_equal, op1=Alu.mult,
                    )
                    nc.vector.tensor_tensor(
                        out=offs[:, cols], in0=offs[:, cols], in1=msk[:], op=Alu.add
                    )

            # rep[p, i, c, :] = feat[4p+i, :] for c = 0..3; built in the RAW
            # region (below) so replication overlaps scatter descriptor-gen.
            rep_h = nc.alloc_sbuf_tensor("rep_raw", [P, IPB, 4, D], f32)
            rep = rep_h[:]

    # --- RAW region. The TileContext epilogue barrier guarantees fB, dur and
    # offs are ready. Copies (DVE: phonemes 0-1, ACT: 2-3) run concurrently
    # with the Pool engine's descriptor generation for the copy-free 1-row
    # scatters; the remaining scatters gate on the copy semaphore. No WAW
    # waits between scatters: Q7 streams desc-gen, SDMA drains overlap.
    fBr = fB[:].rearrange("p (i d) -> p i d", i=IPB)
    cpy_sem = nc.alloc_semaphore("cpy")
    nc.vector.tensor_copy(out=rep[:, 0:2, 0, :], in_=fBr[:, 0:2, :])
    nc.vector.tensor_copy(out=rep[:, 0:2, 1, :], in_=rep[:, 0:2, 0, :])
    nc.vector.tensor_copy(out=rep[:, 0:2, 2:4, :], in_=rep[:, 0:2, 0:2, :]).then_inc(
        cpy_sem, 1
    )
    nc.scalar.copy(out=rep[:, 2:4, 0, :], in_=fBr[:, 2:4, :])
    nc.scalar.copy(out=rep[:, 2:4, 1, :], in_=rep[:, 2:4, 0, :])
    nc.scalar.copy(out=rep[:, 2:4, 2:4, :], in_=rep[:, 2:4, 0:2, :]).then_inc(
        cpy_sem, 1
    )

    scat_sem = nc.alloc_semaphore("scat")
    bregs = {s_: nc.gpsimd.to_reg(MAX_LEN - s_) for s_ in (4, 2, 1)}

    def scat(ci, i, src_ap, s_):
        c = ci * IPB + i
        nc.gpsimd.indirect_dma_start(
            out=out[:, :],
            out_offset=bass.IndirectOffsetOnAxis(ap=offs[:, c : c + 1], axis=0),
            in_=src_ap,
            in_offset=None,
            bounds_check=bregs[s_],
            oob_is_err=False,
        ).then_inc(scat_sem, 16)

    for i in range(IPB):  # 1-row blocks: source is fB, no copy dependency
        scat(4, i, fBr[:, i, :], 1)
    nc.gpsimd.wait_ge(cpy_sem, 2)
    for ci, (s_, moff) in enumerate(BLKS[:4]):  # m-blocks then 2-row tail
        for i in range(IPB):
            scat(ci, i, rep[:, i, 0:s_, :].rearrange("p c d -> p (c d)"), s_)
    nc.gpsimd.wait_ge(scat_sem, 20 * 16)

    nc.compile()
    return nc


def _get_nc():
    if "nc" not in _CACHE:
        _CACHE["nc"] = _build_nc()
    return _CACHE["nc"]


def _run(features, durations, trace=False):
    """features (B,T,D) f32, durations (B,T) i32 -> (out (B,MAX_LEN,D) f32, BassKernelResults)."""
    from concourse.bass_utils import run_bass_kernel_spmd

    nc = _get_nc()
    in_maps = []
    for b in range(B):
        in_maps.append(
            {
                "features": np.ascontiguousarray(features[b]),
                "durations": np.ascontiguousarray(durations[b].reshape(P, IPB)),
            }
        )
    kwargs = {}
    if trace:
        kwargs = dict(trace=True, trace_cores=list(range(B)), stitch_traces=False)
    res = run_bass_kernel_spmd(nc, in_maps, core_ids=list(range(B)), **kwargs)
    outs = np.stack([res.results[b]["out"] for b in range(B)])
    return outs.astype(np.float32, copy=False), res


def kernel(features, durations):
    features = np.asarray(features, dtype=np.float32)
    durations = np.asarray(durations, dtype=np.int32)
    outs, _ = _run(features, durations, trace=False)
    return outs


if __name__ == "__main__":
    feats = np.random.randn(B, T, D).astype(np.float32)
    durs = np.random.randint(0, 16, size=(B, T)).astype(np.int32)
    out = kernel(feats, durs)
    print("out", out.shape, out.dtype)


# revision 23
# speedup vs baseline: 7.8909x; 1.1127x over previous
"""Duration-based length regulation (KittenTTS LengthRegulator) on 8 trn2 NeuronCores.

For each batch b (one per core): phoneme t's feature row is repeated
clamp(durations[b,t],1) times along the frame axis; frames are zero-padded to
MAX_LEN = T*15 (outputs arrive pre-zeroed from the runner, so padding rows are
simply left unwritten... except sink rows, sliced off host-side).

Device strategy (per core, batch-parallel across 8 cores):
  Partition p owns phonemes 4p..4p+3 (IPB=4), loaded as one contiguous DMA.
  Cumsum: per-partition DVE scan + strict-lower-triangular ones matmul on PE
  for the cross-partition exclusive prefix (no host-side index math).
  Expand via 20 single-rank indirect scatters ({4,4,4,2,1}-row blocks x 4
  phoneme slots, dur = 4*q + binary tail; masked slots pushed past
  bounds_check and skipped), issued in a RAW bass region after TileContext:
  without Tile's conservative WAW waits between them, Q7 streams descriptor
  generation back-to-back and the SDMA drains overlap -- the v1 kernel lost
  ~50us to per-call completion stalls here. Write traffic ~= live bytes.
"""

import sys

import numpy as np

if "/opt/trn_rl_repo" not in sys.path:
    sys.path.insert(0, "/opt/trn_rl_repo")

B, T, D = 8, 512, 512
MAX_DUR = 15
MAX_LEN = T * MAX_DUR  # 7680
P = 128
IPB = T // P  # 4 phonemes per partition
BLKS = [(4, 0), (4, 4), (4, 8), (2, None), (1, None)]  # (rows, m-offset)
OOB = 1 << 20

_CACHE = {}


def _build_nc():
    from concourse import bass, mybir
    from concourse.bacc import Bacc
    from concourse.tile import TileContext

    f32, i32, i16 = mybir.dt.float32, mybir.dt.int32, mybir.dt.int16
    Alu = mybir.AluOpType

    nc = Bacc()
    feats = nc.declare_dram_parameter("features", [T, D], f32, isOutput=False)
    durs = nc.declare_dram_parameter("durations", [P, IPB], i32, isOutput=False)
    out = nc.declare_dram_parameter("out", [MAX_LEN, D], f32, isOutput=True)
    
    with TileContext(nc) as tc:
        with (
            tc.tile_pool(name="sbuf", bufs=1) as sb,
            tc.tile_pool(name="psum", bufs=1, space="PSUM") as pp,
        ):
            # --- durations [128, 4]: dur[p, i] = durations[4p+i], clamped >= 1
            dur = sb.tile([P, IPB], i32, tag="dur")
            nc.sync.dma_start(out=dur[:], in_=durs[:, :])
            nc.vector.tensor_scalar_max(out=dur[:], in0=dur[:], scalar1=1)

            # --- features: fB[p, i*D:(i+1)*D] = feat[4p+i, :]
            fB_h = nc.alloc_sbuf_tensor("fB_raw", [P, IPB * D], f32)
            fB = fB_h
            nc.sync.dma_start(
                out=fB[:],
                in_=feats[:, :].rearrange("(p i) d -> p (i d)", p=P),
            )
            fBv = fB[:].rearrange("p (i d) -> p i d", i=IPB)

            # dummy ACT op: forces the activation table load into the preamble
            # instead of stalling the first raw-region scalar copy
            warm = sb.tile([P, 1], f32, tag="warm")
            nc.vector.memset(warm[:], 0.0)
            nc.scalar.copy(out=warm[:], in_=warm[:])

            # --- strict-lower-triangular ones [128, 128]: ltri[k, p] = (p > k)
            iota_f = sb.tile([P, P], i32, tag="iota_f")
            nc.gpsimd.iota(out=iota_f[:], pattern=[[1, P]], base=0, channel_multiplier=0)
            iota_p = sb.tile([P, 1], i32, tag="iota_p")
            nc.gpsimd.iota(out=iota_p[:], pattern=[[1, 1]], base=0, channel_multiplier=1)
            ltri = sb.tile([P, P], f32, tag="ltri")
            nc.vector.tensor_tensor(
                out=ltri[:],
                in0=iota_f[:],
                in1=iota_p[:, 0:1].to_broadcast([P, P]),
                op=Alu.is_gt,
            )

            # --- within-partition inclusive scan of the 4 durations
            scan = sb.tile([P, IPB], i32, tag="scan")
            nc.vector.tensor_tensor_scan(
                out=scan[:],
                data0=dur[:],
                data1=dur[:],
                initial=0.0,
                op0=Alu.add,
                op1=Alu.bypass,
            )

            # --- cross-partition exclusive prefix of per-partition totals
            sf = sb.tile([P, 1], f32, tag="sf")
            nc.vector.tensor_copy(out=sf[:], in_=scan[:, IPB - 1 : IPB])
            pre_ps = pp.tile([P, 1], f32, tag="pre_ps")
            nc.tensor.matmul(pre_ps[:], ltri[:], sf[:], start=True, stop=True)
            pre = sb.tile([P, 1], i32, tag="pre")
            nc.vector.tensor_copy(out=pre[:], in_=pre_ps[:])

            # exc[p, i] = global exclusive cumsum at phoneme 4p+i
            exc = sb.tile([P, IPB], i32, tag="exc")
            nc.vector.tensor_tensor(out=exc[:], in0=scan[:], in1=dur[:], op=Alu.subtract)
            nc.vector.tensor_tensor(
                out=exc[:], in0=exc[:], in1=pre[:, 0:1].to_broadcast([P, IPB]), op=Alu.add
            )

            # --- scatter offsets [128, 20], col ci*4+i. Classes:
            # m-blocks (4 rows at exc+4m, live iff dur >= 4(m+1)) then the
            # binary tail: 2-row at exc+(dur&~3) iff dur&2, 1-row at
            # exc+(dur&~1) iff dur&1. Masked cols pushed to OOB.
            offs_h = nc.alloc_sbuf_tensor("offs_raw", [P, 20], i32)
            offs = offs_h[:]
            hi = sb.tile([P, IPB], i32, tag="hi")
            msk = sb.tile([P, IPB], i32, tag="msk")
            for ci, (s_, moff) in enumerate(BLKS):
                cols = slice(ci * IPB, (ci + 1) * IPB)
                if moff is not None:  # 4-row block m = moff//4
                    nc.vector.tensor_scalar(
                        out=msk[:], in0=dur[:], scalar1=4 + moff, scalar2=OOB - moff,
                        op0=Alu.is_lt, op1=Alu.mult,
                    )
                    nc.vector.tensor_scalar(
                        out=msk[:], in0=msk[:], scalar1=moff, scalar2=None, op0=Alu.add
                    )
                    nc.vector.tensor_tensor(
                        out=offs[:, cols], in0=msk[:], in1=exc[:], op=Alu.add
                    )
                else:  # binary tail block of s_ rows
                    nc.vector.tensor_scalar(
                        out=hi[:], in0=dur[:], scalar1=-(2 * s_), scalar2=None,
                        op0=Alu.bitwise_and,
                    )
                    nc.vector.tensor_tensor(
                        out=offs[:, cols], in0=hi[:], in1=exc[:], op=Alu.add
                    )
                    nc.vector.tensor_scalar(
                        out=msk[:], in0=dur[:], scalar1=s_, scalar2=None,
                        op0=Alu.bitwise_and,
                    )
                    nc.vector.tensor_scalar(
                        out=msk[:], in0=msk[:], scalar1=0, scalar2=OOB,
                        op0=Alu.is_equal, op1=Alu.mult,
                    )
                    nc.vector.tensor_tensor(
                        out=offs[:, cols], in0=offs[:, cols], in1=msk[:], op=Alu.add
                    )

            # rep[p, i, c, :] = feat[4p+i, :] for c = 0..3; built in the RAW
            # region (below) so replication overlaps scatter descriptor-gen.
            rep_h = nc.alloc_sbuf_tensor("rep_raw", [P, IPB, 4, D], f32)
            rep = rep_h[:]

    # --- RAW region. The TileContext epilogue barrier guarantees fB, dur and
    # offs are ready. Copies (DVE: phonemes 0-1, ACT: 2-3) run concurrently
    # with the Pool engine's descriptor generation for the copy-free 1-row
    # scatters; the remaining scatters gate on the copy semaphore. No WAW
    # waits between scatters: Q7 streams desc-gen, SDMA drains overlap.
    fBr = fB[:].rearrange("p (i d) -> p i d", i=IPB)
    cpy_sem = nc.alloc_semaphore("cpy")
    for c in range(3):
        nc.vector.tensor_copy(out=rep[:, 0:2, c, :], in_=fBr[:, 0:2, :])
        nc.scalar.copy(out=rep[:, 2:4, c, :], in_=fBr[:, 2:4, :])
    nc.vector.tensor_copy(out=rep[:, 0:2, 3, :], in_=fBr[:, 0:2, :]).then_inc(
        cpy_sem, 1
    )
    nc.scalar.copy(out=rep[:, 2:4, 3, :], in_=fBr[:, 2:4, :]).then_inc(cpy_sem, 1)

    scat_sem = nc.alloc_semaphore("scat")
    bregs = {s_: nc.gpsimd.to_reg(MAX_LEN - s_) for s_ in (4, 2, 1)}

    def scat(ci, i, src_ap, s_):
        c = ci * IPB + i
        nc.gpsimd.indirect_dma_start(
            out=out[:, :],
            out_offset=bass.IndirectOffsetOnAxis(ap=offs[:, c : c + 1], axis=0),
            in_=src_ap,
            in_offset=None,
            bounds_check=bregs[s_],
            oob_is_err=False,
        ).then_inc(scat_sem, 16)

    for i in range(IPB):  # 1-row blocks: source is fB, no copy dependency
        scat(4, i, fBr[:, i, :], 1)
    nc.gpsimd.wait_ge(cpy_sem, 2)
    for ci, (s_, moff) in enumerate(BLKS[:4]):  # m-blocks then 2-row tail
        for i in range(IPB):
            scat(ci, i, rep[:, i, 0:s_, :].rearrange("p c d -> p (c d)"), s_)
    nc.gpsimd.wait_ge(scat_sem, 20 * 16)

    nc.compile()
    return nc


def _get_nc():
    if "nc" not in _CACHE:
        _CACHE["nc"] = _build_nc()
    return _CACHE["nc"]


def _run(features, durations, trace=False):
    """features (B,T,D) f32, durations (B,T) i32 -> (out (B,MAX_LEN,D) f32, BassKernelResults)."""
    from concourse.bass_utils import run_bass_kernel_spmd

    nc = _get_nc()
    in_maps = []
    for b in range(B):
        in_maps.append(
            {
                "features": np.ascontiguousarray(features[b]),
                "durations": np.ascontiguousarray(durations[b].reshape(P, IPB)),
            }
        )
    kwargs = {}
    if trace:
        kwargs = dict(trace=True, trace_cores=list(range(B)), stitch_traces=False)
    res = run_bass_kernel_spmd(nc, in_maps, core_ids=list(range(B)), **kwargs)
    outs = np.stack([res.results[b]["out"] for b in range(B)])
    return outs.astype(np.float32, copy=False), res


def kernel(features, durations):
    features = np.asarray(features, dtype=np.float32)
    durations = np.asarray(durations, dtype=np.int32)
    outs, _ = _run(features, durations, trace=False)
    return outs


if __name__ == "__main__":
    feats = np.random.randn(B, T, D).astype(np.float32)
    durs = np.random.randint(0, 16, size=(B, T)).astype(np.int32)
    out = kernel(feats, durs)
    print("out", out.shape, out.dtype)
